# revision 1
# baseline (speedup 1.0000x reference)
"""Trainium2 Bass kernel for nn_Attention_90125593739547.

Full-input contract: kernel(**inputs) takes the unsharded numpy inputs and
returns the full [S, B, D] output. Internally:
  - 8 NeuronCores, core c handles batch b = c // 4 and 4 heads (c % 4).
  - Softmax algebra moves biases off the TensorE: the k-bias shifts all
    logits of a softmax row equally (dropped), the v-bias and output bias
    are linear post-terms (added on host), only the q-bias survives (one
    per-partition DVE add at evacuation).
  - Per-core program (bf16 matmuls, optional fp8e4m3 DoubleRow attn@V):
      kT/qT = W.T @ x          [128 (2 heads x 64), 2048] bf16, N=2048 chains
      V2    = x @ Wv stored per t-tile-pair in a DoubleRow-folded layout
              [128, 4h * 2j * 68]; column 64 of each 68-block is memset to 1
              so the PV matmul also accumulates the softmax denominator.
      per head, per q-chunk of 1024:
        sc  = kT_h.T @ qT_h per t-pair          [128, 2 * 1024] PSUM
        pT  = exp(SCALE * sc + C)               one ScalarE op; C keeps the
                                                fp8 values in normal range
                                                and cancels in the ratio
        pv += V2_pair.T @ pT  (DoubleRow K=256) [65, 1024]; row 64 = sum p
        OT  = pv[0:64] * recip(pv[64])          normalization deferred off
                                                the PSUM drain path
      y_partial = OT.T @ Wp                     [2048, 1024] bf16 out
  - Host sums the 4 per-head-group partials per batch and adds bv@Wp + bp.
  - V~ production, the m=1 K/Q chains and the qc0 projection are interleaved
    into the PE slots of the ScalarE-bound attention stream.
"""
import sys
sys.path.insert(0, '/opt/trn_rl_repo')
import numpy as np
from contextlib import ExitStack

S, B, D = 2048, 2, 1024
H, HD = 16, 64
SCALE = 1.0 / (HD ** 0.5)
P = 128
N_CORES = 8
CORES_PER_B = 4
NH = H // CORES_PER_B          # heads per core = 4
HDL = NH * HD                  # local head width = 256
CSHIFT = 2.75                  # exp shift: keeps p' in fp8e4m3 normal range
JVW = 80                       # V2 j-block stride: DoubleRow needs step%16==0
HVW = 2 * JVW                  # per-head V2 stride = 160
NV = NH * HVW                  # V2 row width = 640
QC = 512                       # q-chunk per attention stripe

USE_FP8_PV = True
DEBUG_DUMP = False

_cache = {}


def _build(fp8=USE_FP8_PV, reps=1):
    import concourse.bacc as bacc
    import concourse.mybir as mybir
    from concourse import tile

    nc = bacc.Bacc("TRN2", target_bir_lowering=False, debug=False,
                   num_devices=N_CORES)

    F32 = mybir.dt.float32
    BF16 = mybir.dt.bfloat16
    x = nc.dram_tensor("x", [D, S], BF16, kind="ExternalInput")
    wkqv = nc.dram_tensor("wkqv", [D, 3 * HDL], BF16, kind="ExternalInput")
    bq = nc.dram_tensor("bq", [P, 2], F32, kind="ExternalInput")
    wp = nc.dram_tensor("wp", [HDL, D], BF16, kind="ExternalInput")
    y = nc.dram_tensor("y", [S, D], BF16, kind="ExternalOutput")
    dbg = None
    if DEBUG_DUMP:
        dbg = dict(
            csh=nc.dram_tensor("dbg_csh", [P, 1], F32, kind="ExternalOutput"),
            v2=nc.dram_tensor("dbg_v2", [P, NH * HVW], mybir.dt.float8e4 if fp8 else BF16,
                              kind="ExternalOutput"),
            ot=nc.dram_tensor("dbg_ot", [P, S], BF16, kind="ExternalOutput"),
            kt=nc.dram_tensor("dbg_kt", [P, S], BF16, kind="ExternalOutput"),
            pv=nc.dram_tensor("dbg_pv", [65, QC], F32, kind="ExternalOutput"),
            pt=nc.dram_tensor("dbg_pt", [P, 2 * QC], BF16, kind="ExternalOutput"))

    with tile.TileContext(nc) as tc, ExitStack() as octx:
        if reps > 1:
            octx.enter_context(tc.For_i(0, reps))
        with ExitStack() as ctx:
            _body(nc, tc, ctx, mybir, fp8, x, wkqv, bq, wp, y, dbg)
    nc.compile()
    return nc


def _body(nc, tc, ctx, mybir, fp8, x, wkqv, bq, wp, y, dbg=None):
    F32 = mybir.dt.float32
    BF16 = mybir.dt.bfloat16
    P_DT = mybir.dt.float8e4 if fp8 else BF16
    AF = mybir.ActivationFunctionType
    n_d, n_t = D // P, S // P
    n_qc, n_r = S // QC, n_t // 2

    # ---------------- persistent SBUF ----------------
    const = ctx.enter_context(tc.tile_pool(name="const", bufs=1))
    xb = [const.tile([P, S], BF16, tag=f"x{d}", name=f"x{d}") for d in range(n_d)]
    wkqv_sb = [const.tile([P, 3 * HDL], BF16, tag=f"wkqv{d}", name=f"wkqv{d}")
               for d in range(n_d)]
    wk_sb = [t[:, 0:HDL] for t in wkqv_sb]
    wq_sb = [t[:, HDL:2 * HDL] for t in wkqv_sb]
    wv_sb = [t[:, 2 * HDL:3 * HDL] for t in wkqv_sb]
    bq_sb = const.tile([P, 2], F32, tag="bq", name="bq")
    wp_sb = [const.tile([P, D], BF16, tag=f"wp{m}", name=f"wp{m}") for m in range(2)]
    kT = [const.tile([P, S], BF16, tag=f"kT{m}", name=f"kT{m}") for m in range(2)]
    qT = [const.tile([P, S], BF16, tag=f"qT{m}", name=f"qT{m}") for m in range(2)]
    V2 = [const.tile([P, NV], P_DT, tag=f"V2{r}", name=f"V2{r}") for r in range(n_r)]
    OT = [const.tile([P, S], BF16, tag=f"OT{m}", name=f"OT{m}") for m in range(2)]
    work = ctx.enter_context(tc.tile_pool(name="work", bufs=1))
    ystream = ctx.enter_context(tc.tile_pool(name="ystream", bufs=4))
    rc_pool = ctx.enter_context(tc.tile_pool(name="rc", bufs=1))

    # ---------------- DMA in ----------------
    # The HWDGE costs ~630ns per DMA instruction, the movers run ~360GB/s:
    # keep transfers >= 256KB and the instruction count low.  Weights for
    # k/q/v travel as one 192KB transfer per d-block; x as half-tiles so the
    # kq chains and V~ tiles can chase the stream.
    def dma(out, in_):
        # all input DMAs ride the SP queue: a dma_start on the Activation
        # queue costs ~0.6us of ScalarE sequencer time, which would delay
        # the exp stream behind the descriptor writes
        nc.sync.dma_start(out, in_)

    for d in range(n_d):
        dma(wkqv_sb[d][:], wkqv[d * P:(d + 1) * P, :])
    for d in range(n_d):
        dma(xb[d][:, 0:1024], x[d * P:(d + 1) * P, 0:1024])
    dma(bq_sb[:], bq[:, :])
    for d in range(n_d):
        dma(xb[d][:, 1024:S], x[d * P:(d + 1) * P, 1024:S])
    for m in range(2):
        dma(wp_sb[m][:], wp[m * P:(m + 1) * P, :])

    # ones columns of V2 (softmax denominator rows), written once
    for r in range(n_r):
        col = V2[r][:, :].rearrange("p (h c) -> p h c", h=NH)
        for j in range(2):
            nc.vector.memset(col[:, :, j * JVW + 64:j * JVW + 65], 1.0)
    csh = const.tile([P, 1], F32, tag="csh", name="csh")
    nc.gpsimd.memset(csh[:], CSHIFT)

    # ---------------- PSUM pools (16KB/partition = 8 banks) ----------------
    sc_pool = ctx.enter_context(tc.tile_pool(name="sc", bufs=1, space="PSUM"))    # 2x2 banks
    pv_pool = ctx.enter_context(tc.tile_pool(name="pv", bufs=1, space="PSUM"))    # 1 bank
    chain = ctx.enter_context(tc.tile_pool(name="chain", bufs=1, space="PSUM"))   # 3 banks

    def kq_chunk(dst, wsb, m, lo, is_q):
        ps = chain.tile([P, 512], F32, tag="kq", name="kq", bufs=2)
        for d in range(n_d):
            nc.tensor.matmul(ps[:], wsb[d][:, m * P:(m + 1) * P],
                             xb[d][:, lo:lo + 512],
                             start=(d == 0), stop=(d == n_d - 1))
        if is_q:
            nc.vector.tensor_scalar(dst[m][:, lo:lo + 512], ps[:],
                                    bq_sb[:, m:m + 1], None,
                                    op0=mybir.AluOpType.add)
        else:
            nc.vector.tensor_copy(dst[m][:, lo:lo + 512], ps[:])

    def v_tile(tt):
        """V~ for t-tile tt -> folded slot j=tt%2 of pair tile V2[tt//2]."""
        vp = chain.tile([P, 512], F32, tag="kq", name="vp", bufs=2)
        for d in range(n_d):
            nc.tensor.matmul(vp[:, 0:HDL], xb[d][:, tt * P:(tt + 1) * P],
                             wv_sb[d][:], start=(d == 0), stop=(d == n_d - 1))
        j = tt % 2
        dst = (V2[tt // 2][:, :]
               .rearrange("p (h c) -> p h c", h=NH)[:, :, j * JVW:j * JVW + 64])
        nc.vector.tensor_copy(
            dst, vp[:, 0:HDL].rearrange("p (h c) -> p h c", h=NH))

    # ---- head phase: only the first k/q chunks.  Later k chunks and all of
    # V~ chase the x column-chunk DMAs inside the first attention stripe.
    kq_chunk(kT, wk_sb, 0, 0, False)
    kq_chunk(qT, wq_sb, 0, 0, True)

    # ---------------- attention stripes ----------------
    # Both heads of an m-pair run interleaved in one stripe: each r-step
    # issues two score groups and two exps, so the ScalarE stream stays fed
    # with half the per-stripe boundary cost.
    def attention_pair(m, qc, interleave=None):
        qlo = qc * QC
        pvs = [pv_pool.tile([65, QC], F32, tag=f"pv{half}", name=f"pv{half}")
               for half in range(2)]
        pTs = [[None] * n_r for _ in range(2)]

        def pv_mm(half, r):
            h = 2 * m + half
            lhsT = (V2[r][:, h * HVW:(h + 1) * HVW]
                    .rearrange("p (j c) -> p j c", j=2)[:, :, 0:65])
            if fp8:
                rhs = pTs[half][r][:, :].rearrange("p (j n) -> p j n", j=2)
                nc.tensor.matmul(pvs[half][:], lhsT, rhs,
                                 perf_mode=mybir.MatmulPerfMode.DoubleRow,
                                 start=(r == 0), stop=(r == n_r - 1))
            else:
                for j in range(2):
                    nc.tensor.matmul(pvs[half][:], lhsT[:, j, :],
                                     pTs[half][r][:, j * QC:(j + 1) * QC],
                                     start=(r == 0 and j == 0),
                                     stop=(r == n_r - 1 and j == 1))

        for r in range(n_r):
            for half in range(2):
                plo = half * 64
                sc = sc_pool.tile([P, 2 * QC], F32, tag="sc", name="sc", bufs=2)
                for j in range(2):
                    nc.tensor.matmul(
                        sc[:, j * QC:(j + 1) * QC],
                        kT[m][plo:plo + 64, (2 * r + j) * P:(2 * r + j + 1) * P],
                        qT[m][plo:plo + 64, qlo:qlo + QC],
                        start=True, stop=True)
                pTs[half][r] = work.tile([P, 2 * QC], P_DT, tag="pT", name="pT",
                                         bufs=6)
                nc.scalar.activation(pTs[half][r][:], sc[:], AF.Exp,
                                     bias=csh[:, 0:1], scale=SCALE)
            if r > 0:
                pv_mm(0, r - 1)
                pv_mm(1, r - 1)
            if interleave is not None:
                interleave(r)    # filler PE work, after the critical ops
        pv_mm(0, n_r - 1)
        pv_mm(1, n_r - 1)
        if dbg is not None and m == 0 and qc == 0:
            stg = rc_pool.tile([65, QC], F32, tag="dbgpv", name="dbgpv")
            nc.vector.tensor_copy(stg[:], pvs[0][:])
            nc.sync.dma_start(dbg["pv"][:, :], stg[:])
            stg2 = rc_pool.tile([P, 2 * QC], BF16, tag="dbgpt", name="dbgpt")
            nc.vector.tensor_copy(stg2[:], pTs[0][n_r - 1][:])
            nc.sync.dma_start(dbg["pt"][:, :], stg2[:])
        for half in range(2):
            plo = half * 64
            pv = pvs[half]
            # drain (releases pv).  The denominator row leaves PSUM via
            # tensor_copy first: reciprocal_approx_fast and partition
            # broadcasts mishandle nonzero base partitions on hardware, so
            # every DVE/Pool op below runs at base partition 0 and the final
            # multiply slices matching partition ranges of both operands.
            den = rc_pool.tile([1, QC], F32, tag="den", name="den", bufs=4)
            nc.vector.tensor_copy(den[:], pv[64:65, :])
            nc.vector.tensor_copy(OT[m][plo:plo + 64, qlo:qlo + QC], pv[0:64, :])
            rc1 = rc_pool.tile([1, QC], F32, tag="rc1", name="rc1", bufs=4)
            nc.vector.reciprocal_approx_fast(rc1[:], den[:])
            rcb = rc_pool.tile([P, QC], F32, tag="rcb", name="rcb", bufs=2)
            nc.gpsimd.partition_broadcast(rcb[:], rc1[0:1, :])
            nc.vector.tensor_tensor(OT[m][plo:plo + 64, qlo:qlo + QC],
                                    OT[m][plo:plo + 64, qlo:qlo + QC],
                                    rcb[plo:plo + 64, :],
                                    op=mybir.AluOpType.mult)

    def proj_steps(qc):
        """Projection of q-chunk qc: one 512-wide output block per step."""
        qlo = qc * QC
        for qt in range(qlo // P, (qlo + QC) // P):
            yt = ystream.tile([P, D], BF16, tag="yt", name="yt")
            for nn in range(0, D, 512):
                ps = chain.tile([P, 512], F32, tag="kq", name="proj", bufs=2)
                for m in range(2):
                    nc.tensor.matmul(ps[:], OT[m][:, qt * P:(qt + 1) * P],
                                     wp_sb[m][:, nn:nn + 512],
                                     start=(m == 0), stop=(m == 1))
                nc.vector.tensor_copy(yt[:, nn:nn + 512], ps[:])
                yield
            nc.sync.dma_start(y[qt * P:(qt + 1) * P, :], yt[:])
        while True:
            yield

    # qc-major stripe order.  (qc0,h0) finishes V~ just-in-time for its own
    # pv accumulation; the m=1 K/Q chains are spread over the (qc0,h1/h2)
    # slack; (qc,h2/h3) produce the next qT m0 chunk; (qc,h0) carries the
    # previous chunk's projection; proj(qc3) is the tail.
    projs = [proj_steps(qc) for qc in range(n_qc)]

    def ilv(table):
        def f(r):
            fn = table.get(r)
            if fn is not None:
                fn()
        return f

    def chase(r):
        # x half-tiles land -> k chunks (sc 2c gates on chunk c) + V~ tiles
        if r == 1:
            kq_chunk(kT, wk_sb, 0, 512, False)
        elif r == 3:
            kq_chunk(kT, wk_sb, 0, 1024, False)
        elif r == 5:
            kq_chunk(kT, wk_sb, 0, 1536, False)
        v_tile(2 * r)
        v_tile(2 * r + 1)
        if r == 6:
            kq_chunk(qT, wq_sb, 0, 512, True)

    attention_pair(0, 0, interleave=chase)
    attention_pair(0, 1, interleave=ilv({
        0: lambda: kq_chunk(qT, wq_sb, 0, 1024, True),
        2: lambda: kq_chunk(qT, wq_sb, 1, 0, True),
        4: lambda: kq_chunk(kT, wk_sb, 1, 0, False),
        6: lambda: kq_chunk(kT, wk_sb, 1, 512, False)}))
    attention_pair(0, 2, interleave=ilv({
        0: lambda: kq_chunk(qT, wq_sb, 0, 1536, True),
        2: lambda: kq_chunk(kT, wk_sb, 1, 1024, False),
        4: lambda: kq_chunk(kT, wk_sb, 1, 1536, False)}))
    attention_pair(0, 3, interleave=ilv({
        0: lambda: kq_chunk(qT, wq_sb, 1, 512, True),
        2: lambda: kq_chunk(qT, wq_sb, 1, 1024, True),
        4: lambda: kq_chunk(qT, wq_sb, 1, 1536, True)}))
    projs = [proj_steps(qc) for qc in range(n_qc)]
    attention_pair(1, 0)
    attention_pair(1, 1, interleave=lambda r: next(projs[0]))
    next(projs[0])           # flush the trailing output DMA
    attention_pair(1, 2, interleave=lambda r: next(projs[1]))
    next(projs[1])
    attention_pair(1, 3, interleave=lambda r: next(projs[2]))
    next(projs[2])
    for _ in range(QC // P * (D // 512) + 1):
        next(projs[n_qc - 1])
    if dbg is not None:
        nc.sync.dma_start(dbg["csh"][:, :], csh[:])
        nc.sync.dma_start(dbg["v2"][:, :], V2[0][:])
        nc.sync.dma_start(dbg["ot"][:, :], OT[0][:])
        nc.sync.dma_start(dbg["kt"][:, :], kT[0][:])


def _get_nc(fp8=USE_FP8_PV, reps=1):
    key = (fp8, reps)
    if key not in _cache:
        _cache[key] = _build(fp8=fp8, reps=reps)
    return _cache[key]


def make_in_maps(inputs, Wkv, bkv, Wq, bq, Wp, bp):
    """Host-side sharding: per-core input dicts (bf16)."""
    import ml_dtypes
    BF = ml_dtypes.bfloat16
    inputs = np.asarray(inputs, dtype=np.float32)
    Wkv = np.asarray(Wkv, dtype=np.float32)
    Wq = np.asarray(Wq, dtype=np.float32)
    bq = np.asarray(bq, dtype=np.float32)
    Wp = np.asarray(Wp, dtype=np.float32)

    in_maps = []
    for c in range(N_CORES):
        b = c // CORES_PER_B
        g = c % CORES_PER_B
        hsl = slice(g * HDL, (g + 1) * HDL)
        wkqv = np.concatenate([
            Wkv[:, hsl], Wq[:, hsl],
            Wkv[:, H * HD + g * HDL: H * HD + (g + 1) * HDL]], axis=1)
        in_maps.append(dict(
            x=np.ascontiguousarray(inputs[:, b, :].T).astype(BF),
            wkqv=np.ascontiguousarray(wkqv).astype(BF),
            bq=np.ascontiguousarray(bq[hsl].reshape(2, P).T),
            wp=np.ascontiguousarray(Wp[hsl, :]).astype(BF)))
    return in_maps


def combine_outputs(results):
    """Host-side unshard: sum the head-group partials per batch."""
    out = np.zeros((S, B, D), np.float32)
    for b in range(B):
        acc = results[b * CORES_PER_B]["y"].astype(np.float32)
        for g in range(1, CORES_PER_B):
            acc += results[b * CORES_PER_B + g]["y"].astype(np.float32)
        out[:, b, :] = acc
    return out


def kernel(inputs, Wkv, bkv, Wq, bq, Wp, bp):
    from concourse.bass_utils import run_bass_kernel_spmd
    nc = _get_nc()
    in_maps = make_in_maps(inputs, Wkv, bkv, Wq, bq, Wp, bp)
    res = run_bass_kernel_spmd(nc, in_maps, list(range(N_CORES)))
    out = combine_outputs(res.results)
    # bias terms hoisted off-device: y += bv @ Wp + bp  (softmax weights sum
    # to one, so the v-bias contributes a constant row through Wp)
    bkv64 = np.asarray(bkv, np.float64)
    bias = (bkv64[H * HD:] @ np.asarray(Wp, np.float64)
            + np.asarray(bp, np.float64)).astype(np.float32)
    out += bias[None, None, :]
    return out



# revision 4
# speedup vs baseline: 1.1758x; 1.1758x over previous
"""Trainium2 Bass kernel for nn_Attention_90125593739547.

Full-input contract: kernel(**inputs) takes the unsharded numpy inputs and
returns the full [S, B, D] output. Internally:
  - 8 NeuronCores, core c handles batch b = c // 4 and 4 heads (c % 4).
  - Softmax algebra moves biases off the TensorE: the k-bias shifts all
    logits of a softmax row equally (dropped), the v-bias and output bias
    are linear post-terms (added on host), only the q-bias survives (one
    per-partition DVE add at evacuation).

Scheduling (v2): the kernel is ScalarE-bound (128 exp ACTIVATEs of
(1024+352)/1.2 ns = 147us); the PE must stay >90% busy so the HAM clock
gate never drops it to 1.2 GHz.  The For_i body is unrolled 2x with
double-buffered K/Q/V sets: each body runs pure attention on the current
set while producing the NEXT iteration's K/Q/V + V2 tiles as evenly
spread PE filler.  A prologue outside the loop produces iteration 0's
set; each body's trailing projection (qc3) is carried into the next
body's first stripe; a flush after the loop emits the last one.

Per-core program (bf16 matmuls, fp8e4m3 DoubleRow attn@V):
  kT/qT = W.T @ x          [128 (2 heads x 64), 2048] bf16, N=2048 chains
  V2    = x @ Wv stored per t-tile-pair in a DoubleRow-folded layout
          [128, 4h * 2j * 68]; column 64 of each 68-block is memset to 1
          so the PV matmul also accumulates the softmax denominator.
  per head-pair (m), per q-chunk of 512:
    sc  = kT_h.T @ qT_h per t-pair          [128, 2 * 512] PSUM
    pT  = exp(SCALE * sc + C)               one ScalarE op; C keeps the
                                            fp8 values in normal range
                                            and cancels in the ratio
    pv += V2_pair.T @ pT  (DoubleRow K=256) [65, 512]; row 64 = sum p
    OT  = pv[0:64] * recip(pv[64])
  y_partial = OT.T @ Wp                     [2048, 1024] bf16 out
Host sums the 4 per-head-group partials per batch and adds bv@Wp + bp.
"""
import sys
sys.path.insert(0, '/opt/trn_rl_repo')
import numpy as np
from contextlib import ExitStack

S, B, D = 2048, 2, 1024
H, HD = 16, 64
SCALE = 1.0 / (HD ** 0.5)
P = 128
N_CORES = 8
CORES_PER_B = 4
NH = H // CORES_PER_B          # heads per core = 4
HDL = NH * HD                  # local head width = 256
CSHIFT = 2.75                  # exp shift: keeps p' in fp8e4m3 normal range
JVW = 80                       # V2 j-block stride: DoubleRow needs step%16==0
HVW = 2 * JVW                  # per-head V2 stride = 160
NV = NH * HVW                  # V2 row width = 640
QC = 512                       # q-chunk per attention stripe

_cache = {}


def _build(reps=1):
    import concourse.bacc as bacc
    import concourse.mybir as mybir
    from concourse import tile

    nc = bacc.Bacc("TRN2", target_bir_lowering=False, debug=False,
                   num_devices=N_CORES)

    F32 = mybir.dt.float32
    BF16 = mybir.dt.bfloat16
    x = nc.dram_tensor("x", [D, S], BF16, kind="ExternalInput")
    wkqv = nc.dram_tensor("wkqv", [D, 3 * HDL], BF16, kind="ExternalInput")
    bq = nc.dram_tensor("bq", [P, 2], F32, kind="ExternalInput")
    wp = nc.dram_tensor("wp", [HDL, D], BF16, kind="ExternalInput")
    y = nc.dram_tensor("y", [S, D], BF16, kind="ExternalOutput")

    with tile.TileContext(nc) as tc:
        k = _Kernel(nc, tc, mybir, x, wkqv, bq, wp, y)
        with ExitStack() as ctx:
            k.alloc(ctx)
            k.prologue()
            if reps == 1:
                k.body(0)
                k.flush(0)
            else:
                assert reps % 2 == 0
                with tc.For_i(0, reps // 2):
                    k.body(0)
                    k.body(1)
                k.flush(1)
    nc.compile()
    return nc


class _Kernel:
    def __init__(self, nc, tc, mybir, x, wkqv, bq, wp, y):
        self.nc, self.tc, self.mybir = nc, tc, mybir
        self.x, self.wkqv, self.bq, self.wp, self.y = x, wkqv, bq, wp, y
        self.n_d = D // P            # 8
        self.n_t = S // P            # 16
        self.n_qc = S // QC          # 4
        self.n_r = self.n_t // 2     # 8

    # ---------------- allocation ----------------
    def alloc(self, ctx):
        nc, tc, mybir = self.nc, self.tc, self.mybir
        F32, BF16 = mybir.dt.float32, mybir.dt.bfloat16
        P_DT = mybir.dt.float8e4
        const = ctx.enter_context(tc.tile_pool(name="const", bufs=1))
        t = const.tile
        self.xb = [[t([P, S], BF16, tag=f"x{s}_{d}", name=f"x{s}_{d}")
                    for d in range(self.n_d)] for s in range(2)]
        self.wkqv_sb = [[t([P, 3 * HDL], BF16, tag=f"wkqv{s}_{d}",
                           name=f"wkqv{s}_{d}") for d in range(self.n_d)]
                        for s in range(2)]
        self.wk_sb = [[w[:, 0:HDL] for w in ws] for ws in self.wkqv_sb]
        self.wq_sb = [[w[:, HDL:2 * HDL] for w in ws] for ws in self.wkqv_sb]
        self.wv_sb = [[w[:, 2 * HDL:3 * HDL] for w in ws] for ws in self.wkqv_sb]
        self.bq_sb = t([P, 2], F32, tag="bq", name="bq")
        self.wp_sb = [[t([P, D], BF16, tag=f"wp{s}_{m}", name=f"wp{s}_{m}")
                       for m in range(2)] for s in range(2)]
        self.kT = [[t([P, S], BF16, tag=f"kT{s}_{m}", name=f"kT{s}_{m}")
                    for m in range(2)] for s in range(2)]
        self.qT = [[t([P, S], BF16, tag=f"qT{s}_{m}", name=f"qT{s}_{m}")
                    for m in range(2)] for s in range(2)]
        self.V2 = [[t([P, NV], P_DT, tag=f"V2{s}_{r}", name=f"V2{s}_{r}")
                    for r in range(self.n_r)] for s in range(2)]
        self.OT = [[t([P, S], BF16, tag=f"OT{s}_{m}", name=f"OT{s}_{m}")
                    for m in range(2)] for s in range(2)]
        self.csh = t([P, 1], F32, tag="csh", name="csh")
        self.work = ctx.enter_context(tc.tile_pool(name="work", bufs=1))
        self.ystream = ctx.enter_context(tc.tile_pool(name="ystream", bufs=4))
        self.rc_pool = ctx.enter_context(tc.tile_pool(name="rc", bufs=1))
        # PSUM: sc 2x2 banks + pv 2 + chain 2 = 8
        self.sc_pool = ctx.enter_context(
            tc.tile_pool(name="sc", bufs=1, space="PSUM"))
        self.pv_pool = ctx.enter_context(
            tc.tile_pool(name="pv", bufs=1, space="PSUM"))
        self.chain = ctx.enter_context(
            tc.tile_pool(name="chain", bufs=1, space="PSUM"))

    # ---------------- DMA ----------------
    def dma_in(self, st):
        """Issue input DMAs filling buffer set `st`.  Order matters: the
        first chains of the consuming body need wkqv + x[:, 0:512]."""
        nc = self.nc
        for d in range(self.n_d):
            nc.sync.dma_start(self.wkqv_sb[st][d][:],
                              self.wkqv[d * P:(d + 1) * P, :])
        for c in range(4):
            for d in range(self.n_d):
                nc.sync.dma_start(self.xb[st][d][:, c * 512:(c + 1) * 512],
                                  self.x[d * P:(d + 1) * P, c * 512:(c + 1) * 512])
        for m in range(2):
            nc.sync.dma_start(self.wp_sb[st][m][:],
                              self.wp[m * P:(m + 1) * P, :])

    # ---------------- chain producers (write set st) ----------------
    def kq_chunk(self, st, dst, wsb, m, lo, is_q):
        nc, mybir = self.nc, self.mybir
        ps = self.chain.tile([P, 512], mybir.dt.float32, tag="kq", name="kq",
                             bufs=2)
        for d in range(self.n_d):
            nc.tensor.matmul(ps[:], wsb[st][d][:, m * P:(m + 1) * P],
                             self.xb[st][d][:, lo:lo + 512],
                             start=(d == 0), stop=(d == self.n_d - 1))
        if is_q:
            nc.vector.tensor_scalar(dst[st][m][:, lo:lo + 512], ps[:],
                                    self.bq_sb[:, m:m + 1], None,
                                    op0=mybir.AluOpType.add)
        else:
            nc.vector.tensor_copy(dst[st][m][:, lo:lo + 512], ps[:])

    def v_tile(self, st, tt):
        """V~ for t-tile tt -> folded slot j=tt%2 of pair tile V2[st][tt//2]."""
        nc, mybir = self.nc, self.mybir
        vp = self.chain.tile([P, 512], mybir.dt.float32, tag="kq", name="vp",
                             bufs=2)
        for d in range(self.n_d):
            nc.tensor.matmul(vp[:, 0:HDL], self.xb[st][d][:, tt * P:(tt + 1) * P],
                             self.wv_sb[st][d][:],
                             start=(d == 0), stop=(d == self.n_d - 1))
        j = tt % 2
        dst = (self.V2[st][tt // 2][:, :]
               .rearrange("p (h c) -> p h c", h=NH)[:, :, j * JVW:j * JVW + 64])
        nc.vector.tensor_copy(
            dst, vp[:, 0:HDL].rearrange("p (h c) -> p h c", h=NH))

    # ---------------- projection (reads OT[st], wp[st]) ----------------
    def proj_steps(self, st, qc):
        """Projection of q-chunk qc: one 512-wide output block per step.
        8 steps total (4 qt-tiles x 2 n-blocks); the output DMA of each
        qt-tile is emitted before the second yield so 8 next() calls emit
        everything."""
        nc, mybir = self.nc, self.mybir
        qlo = qc * QC
        for qt in range(qlo // P, (qlo + QC) // P):
            yt = self.ystream.tile([P, D], mybir.dt.bfloat16, tag="yt",
                                   name="yt")
            for nn in range(0, D, 512):
                ps = self.chain.tile([P, 512], mybir.dt.float32, tag="kq",
                                     name="proj", bufs=2)
                for m in range(2):
                    nc.tensor.matmul(ps[:], self.OT[st][m][:, qt * P:(qt + 1) * P],
                                     self.wp_sb[st][m][:, nn:nn + 512],
                                     start=(m == 0), stop=(m == 1))
                nc.vector.tensor_copy(yt[:, nn:nn + 512], ps[:])
                if nn == 0:
                    yield
            nc.sync.dma_start(self.y[qt * P:(qt + 1) * P, :], yt[:])
            yield

    # ---------------- prologue ----------------
    def prologue(self):
        nc, mybir = self.nc, self.mybir
        self.dma_in(0)
        self.dma_in(1)
        nc.sync.dma_start(self.bq_sb[:], self.bq[:, :])
        # ones columns of V2 (softmax denominator rows), written once: the
        # v_tile copies only touch [:, :64] of each 80-wide j-block.
        for st in range(2):
            for r in range(self.n_r):
                col = self.V2[st][r][:, :].rearrange("p (h c) -> p h c", h=NH)
                for j in range(2):
                    nc.vector.memset(col[:, :, j * JVW + 64:j * JVW + 65], 1.0)
            for m in range(2):
                nc.vector.memset(self.OT[st][m][:], 0.0)
        nc.gpsimd.memset(self.csh[:], CSHIFT)
        # iteration-0 chain set
        for m in range(2):
            for lo in range(0, S, 512):
                self.kq_chunk(0, self.kT, self.wk_sb, m, lo, False)
                self.kq_chunk(0, self.qT, self.wq_sb, m, lo, True)
        for tt in range(self.n_t):
            self.v_tile(0, tt)

    # ---------------- filler schedule ----------------
    def filler(self, st, carry):
        """Per (stripe, r) lists of thunks.  st = set under production
        (the NEXT iteration's buffers); carry = leftover proj steps of
        the previous body (reads OT[1-st]... supplied by caller)."""
        nxt = st
        kq = self.kq_chunk
        vt = self.v_tile
        KT, QT = self.kT, self.qT
        wk, wq = self.wk_sb, self.wq_sb

        def K(m, lo):
            return lambda: kq(nxt, KT, wk, m, lo, False)

        def Q(m, lo):
            return lambda: kq(nxt, QT, wq, m, lo, True)

        def V(tt):
            return lambda: vt(nxt, tt)

        def Pj(gen):
            return lambda: next(gen, None)

        own = [self.proj_steps(1 - nxt, qc) for qc in range(self.n_qc)]
        c = [Pj(carry)] * 8 if carry is not None else []
        p0, p1, p2 = [Pj(own[0])] * 8, [Pj(own[1])] * 8, [Pj(own[2])] * 8
        self.own_proj = own
        # stripe index 0..7 = (m=0 qc0..3), (m=1 qc0..3); 8 r-slots each.
        # Deadlines: kT/qT[m0]+qT lo0 before next body stripe 0; all by
        # body end.  V2 all by body end.  proj(qc) after stripe 4+qc.
        # DMA for set (1-nxt) issues at stripe 6 (consumed 2 bodies later).
        sched = [
            # stripe 0: carry proj (2/r) then first chains (x lands ~7us in)
            [c[0:2], c[2:4], c[4:6], c[6:8],
             [K(0, 0)], [V(0), V(1)], [K(0, 512)], []],
            # stripe 1
            [[K(0, 1024)], [V(2)], [K(0, 1536)], [V(3)],
             [Q(0, 0)], [V(4)], [K(1, 0)], [V(5)]],
            # stripe 2
            [[K(1, 512)], [V(6)], [K(1, 1024)], [V(7)],
             [K(1, 1536)], [V(8)], [Q(1, 0)], [V(9)]],
            # stripe 3
            [[Q(0, 512)], [V(10)], [Q(1, 512)], [V(11)],
             [Q(0, 1024)], [V(12)], [Q(1, 1024)], [V(13)]],
            # stripe 4
            [[Q(0, 1536)], [V(14)], [Q(1, 1536)], [V(15)],
             [], [], [], []],
            # stripe 5: proj(qc0) now available
            [p0[0:1], p0[1:3], p0[3:4], p0[4:5], p0[5:7], p0[7:8],
             [self._dma_thunk(1 - nxt)], []],
            # stripe 6: proj(qc1)
            [p1[0:1], p1[1:3], p1[3:4], p1[4:5], p1[5:7], p1[7:8], [], []],
            # stripe 7: proj(qc2)
            [p2[0:1], p2[1:3], p2[3:4], p2[4:5], p2[5:7], p2[7:8], [], []],
        ]
        return sched

    def _dma_thunk(self, st):
        return lambda: self.dma_in(st)

    # ---------------- attention stripe ----------------
    def attention_pair(self, st, m, qc, slots):
        nc, mybir = self.nc, self.mybir
        AF = self.mybir.ActivationFunctionType
        F32 = mybir.dt.float32
        P_DT = mybir.dt.float8e4
        n_r = self.n_r
        qlo = qc * QC
        kT, qT, V2, OT = (self.kT[st], self.qT[st], self.V2[st], self.OT[st])
        pvs = [self.pv_pool.tile([65, QC], F32, tag=f"pv{half}",
                                 name=f"pv{half}") for half in range(2)]
        pTs = [[None] * n_r for _ in range(2)]

        def pv_mm(half, r):
            h = 2 * m + half
            lhsT = (V2[r][:, h * HVW:(h + 1) * HVW]
                    .rearrange("p (j c) -> p j c", j=2)[:, :, 0:65])
            rhs = pTs[half][r][:, :].rearrange("p (j n) -> p j n", j=2)
            nc.tensor.matmul(pvs[half][:], lhsT, rhs,
                             perf_mode=mybir.MatmulPerfMode.DoubleRow,
                             start=(r == 0), stop=(r == n_r - 1))

        for r in range(n_r):
            for half in range(2):
                plo = half * 64
                sc = self.sc_pool.tile([P, 2 * QC], F32, tag="sc", name="sc",
                                       bufs=2)
                for j in range(2):
                    nc.tensor.matmul(
                        sc[:, j * QC:(j + 1) * QC],
                        kT[m][plo:plo + 64, (2 * r + j) * P:(2 * r + j + 1) * P],
                        qT[m][plo:plo + 64, qlo:qlo + QC],
                        start=True, stop=True)
                pTs[half][r] = self.work.tile([P, 2 * QC], P_DT, tag="pT",
                                              name="pT", bufs=6)
                nc.scalar.activation(pTs[half][r][:], sc[:], AF.Exp,
                                     bias=self.csh[:, 0:1], scale=SCALE)
            if r > 0:
                pv_mm(0, r - 1)
                pv_mm(1, r - 1)
            for thunk in slots[r]:
                thunk()
        pv_mm(0, n_r - 1)
        pv_mm(1, n_r - 1)
        for half in range(2):
            plo = half * 64
            pv = pvs[half]
            # drain (releases pv).  reciprocal_approx_fast and partition
            # broadcasts mishandle nonzero base partitions on hardware, so
            # every DVE/Pool op below runs at base partition 0 and the final
            # multiply slices matching partition ranges of both operands.
            den = self.rc_pool.tile([1, QC], F32, tag="den", name="den", bufs=4)
            nc.vector.tensor_copy(den[:], pv[64:65, :])
            nc.vector.tensor_copy(OT[m][plo:plo + 64, qlo:qlo + QC], pv[0:64, :])
            rc1 = self.rc_pool.tile([1, QC], F32, tag="rc1", name="rc1", bufs=4)
            nc.vector.reciprocal_approx_fast(rc1[:], den[:])
            rcb = self.rc_pool.tile([P, QC], F32, tag="rcb", name="rcb", bufs=2)
            nc.gpsimd.partition_broadcast(rcb[:], rc1[0:1, :])
            nc.vector.tensor_tensor(OT[m][plo:plo + 64, qlo:qlo + QC],
                                    OT[m][plo:plo + 64, qlo:qlo + QC],
                                    rcb[plo:plo + 64, :],
                                    op=self.mybir.AluOpType.mult)

    # ---------------- body ----------------
    def body(self, cur):
        nxt = 1 - cur
        # carry: previous body's proj(qc3) on the OTHER set.  On the first
        # pass this reads zeroed OT (prologue memset) and writes a harmless
        # zero y[qc3] block that later iterations overwrite; the flush
        # after the loop emits the final correct one.
        carry = self.proj_steps(1 - cur, 3)
        sched = self.filler(nxt, carry)
        si = 0
        for m in range(2):
            for qc in range(self.n_qc):
                self.attention_pair(cur, m, qc, sched[si])
                si += 1

    def flush(self, last_cur):
        gen = self.proj_steps(last_cur, 3)
        for _ in range(8):
            next(gen, None)


def _get_nc(reps=1):
    if reps not in _cache:
        _cache[reps] = _build(reps=reps)
    return _cache[reps]


def make_in_maps(inputs, Wkv, bkv, Wq, bq, Wp, bp):
    """Host-side sharding: per-core input dicts (bf16)."""
    import ml_dtypes
    BF = ml_dtypes.bfloat16
    inputs = np.asarray(inputs, dtype=np.float32)
    Wkv = np.asarray(Wkv, dtype=np.float32)
    Wq = np.asarray(Wq, dtype=np.float32)
    bq = np.asarray(bq, dtype=np.float32)
    Wp = np.asarray(Wp, dtype=np.float32)

    in_maps = []
    for c in range(N_CORES):
        b = c // CORES_PER_B
        g = c % CORES_PER_B
        hsl = slice(g * HDL, (g + 1) * HDL)
        wkqv = np.concatenate([
            Wkv[:, hsl], Wq[:, hsl],
            Wkv[:, H * HD + g * HDL: H * HD + (g + 1) * HDL]], axis=1)
        in_maps.append(dict(
            x=np.ascontiguousarray(inputs[:, b, :].T).astype(BF),
            wkqv=np.ascontiguousarray(wkqv).astype(BF),
            bq=np.ascontiguousarray(bq[hsl].reshape(2, P).T),
            wp=np.ascontiguousarray(Wp[hsl, :]).astype(BF)))
    return in_maps


def combine_outputs(results):
    """Host-side unshard: sum the head-group partials per batch."""
    out = np.zeros((S, B, D), np.float32)
    for b in range(B):
        acc = results[b * CORES_PER_B]["y"].astype(np.float32)
        for g in range(1, CORES_PER_B):
            acc += results[b * CORES_PER_B + g]["y"].astype(np.float32)
        out[:, b, :] = acc
    return out


def kernel(inputs, Wkv, bkv, Wq, bq, Wp, bp):
    from concourse.bass_utils import run_bass_kernel_spmd
    nc = _get_nc()
    in_maps = make_in_maps(inputs, Wkv, bkv, Wq, bq, Wp, bp)
    res = run_bass_kernel_spmd(nc, in_maps, list(range(N_CORES)))
    out = combine_outputs(res.results)
    # bias terms hoisted off-device: y += bv @ Wp + bp  (softmax weights sum
    # to one, so the v-bias contributes a constant row through Wp)
    bkv64 = np.asarray(bkv, np.float64)
    bias = (bkv64[H * HD:] @ np.asarray(Wp, np.float64)
            + np.asarray(bp, np.float64)).astype(np.float32)
    out += bias[None, None, :]
    return out
